# revision 37
# baseline (speedup 1.0000x reference)
"""MoE experts kernel for TRN2, expert-parallel over 8 NeuronCores.

Reference computation (T=4096, E=8, H=1024, Q=1024):
    gate_up = einsum('th,ehq->teq', x, gate_up_proj)      # (T, E, 2Q)
    gate, up = split(gate_up, 2, axis=-1)
    hidden = silu(gate) * up                              # (T, E, Q)
    expert_outputs = einsum('teq,eqh->teh', hidden, down_proj)
    out = einsum('teh,te->th', expert_outputs, routing_weights)

Sharding: expert-parallel. Core e computes its expert's full contribution
r[:, e] * (silu(x @ Wgu_gate) * (x @ Wgu_up)) @ Wdn  for all T tokens,
entirely in feature-major layout (features on partitions, tokens on the
free axis) so no on-device transposes are needed; the host sums the 8
partial outputs (the expert-parallel all-reduce) and transposes back.

Per-core cost model (measured; ~352us total = 16.5 front + 331 stream
+ 5 drain):
  - 1536 bf16 matmuls of [128 contraction x 512 moving] at ~216ns each
    (512 cycles at the 2.37GHz sustained PE clock, zero per-instruction
    overhead -- bigger moving tiles would save nothing) = 332us PE busy.
    fp8 is no help twice over: its quantization error (3.8-6.5%
    end-to-end) eats the 2e-2 gate's margin, AND DoubleRow measures at
    most ~1.44x, so one residual-compensation matmul already makes it a
    net loss vs bf16.
  - Each dma_start costs ~2us fixed (completion receipt) + transfer,
    serialized per queue; per-queue throughput ~115GB/s warm, and the
    whole DMA subsystem is capped at ~200GB/s aggregate for the first
    ~10us regardless of queue count or descriptor sizes (measured
    identically across 2-way/3-way splits and fine/coarse pieces). So
    the 1.5MB the first k-sweep needs (x chunk 0 + slabs 0/1) cannot
    land before ~16.5us: prologue 7.6 + issue 0.7 + 1.5MB/200GB/s + 2
    fixed. The boot blob packs exactly that as ONE fat DMA per HWDGE
    queue; everything else is host-packed for 4-16KB contiguous rows
    and scheduled against the PE's consumption deadlines across the
    three queues (SP=nc.sync, ACT=nc.scalar, SWDGE=nc.gpsimd; SWDGE is
    slower and gets only far-deadline traffic).
  - The PE clock ramps over ~8 matmuls (~427ns each) after idling;
    dummy matmuls during the unavoidable initial DMA wait pay that cost
    off the critical path. A mid-stream starve >1us re-ramps (costly
    twice), so every weight pair must beat its deadline.
  - An external ~50-160ns hiccup hits the PE every 10.79us (~1.7us
    total, phase-random): unfixable, and the source of most run-to-run
    variance. Back-to-back runs throttle the chip to ~2.0GHz (matmul
    p50 259ns) -- cool ~2min between measurements.
  - ~7.5us fixed framework prologue before any DMA issues; ~1.8us
    epilogue barrier after the last store's semaphore. Drain floor
    ~4.9us: last mul + 2us DMA fixed + 128KB + epilogue (SWDGE for the
    final piece has +-2us latency variance; keep it on SP).
"""

import sys

for _p in ("/opt/trn_rl_repo", "/root/.axon_site/_ro/trn_rl_repo"):
    if _p not in sys.path:
        sys.path.insert(0, _p)

import numpy as np

T, E, H, Q = 4096, 8, 1024, 1024
P = 128          # partitions
TC = 512         # token chunk (= one PSUM bank of fp32)
NT = T // TC     # 8 token chunks
KH = H // P      # 8 contraction tiles for the gate_up matmul
KQ = Q // P      # 8 contraction tiles for the down matmul
NH = H // P      # 8 output-feature tiles
NS = 2 * Q // P  # 16 gate_up weight slabs (gate qi / up qi interleaved)
N_WARM = 18      # PE clock-warmup dummies covering the first-DMA wait
BOOT = KH * TC // 2 + KH * P  # 3072: half of x chunk 0 + one slab, per queue

_CACHED = None


def _split_waits(nc, max_waits=1):
    """Walrus codegen for several TRN2 ISA structs accepts only one sync-wait
    per instruction ("Too many sync wait commands"). Splitting is safe: a
    same-engine NoOp earlier in the (FIFO) stream carrying the extra waits
    blocks the stream at the same point the original multi-wait would have."""
    import concourse.mybir as mybir

    for f in nc.m.functions:
        for blk in f.blocks:
            newlist, changed = [], False
            for inst in blk.instructions:
                si = inst.sync_info
                if si is not None and si.on_wait and len(si.on_wait) > max_waits:
                    extra = si.on_wait[:-max_waits]
                    keep = si.on_wait[-max_waits:]
                    inst.sync_info = mybir.SyncInfo(
                        on_wait=list(keep), on_update=list(si.on_update or [])
                    )
                    for j, w in enumerate(extra):
                        nop = mybir.InstNoOp(
                            name=f"{inst.name}-wn{j}", engine=inst.engine
                        )
                        nop.sync_info = mybir.SyncInfo(on_wait=[w], on_update=[])
                        newlist.append(nop)
                    changed = True
                newlist.append(inst)
            if changed:
                blk.instructions = newlist


def _build():
    import concourse.bass as bass
    import concourse.mybir as mybir
    import concourse.tile as tile

    nc = bass.Bass("TRN2", target_bir_lowering=False, debug=False, num_devices=E)

    f32 = mybir.dt.float32
    # bf16: same PE rate as fp32r (1 cycle/row for moving >= 256) but half
    # the HBM traffic and half-width weight loads; quantization adds ~0.3%
    # relative error, well inside the 2e-2 gate.
    bf16 = mybir.dt.bfloat16

    # All DRAM layouts are host-packed so each DMA reads big contiguous
    # per-partition rows (see _make_in_maps):
    #   w_gu: quad-major [quad, P, slab-in-quad, KH, P]  -> 8KB rows/quad
    #   w_dn: [P, KQ, H]                                 -> 16KB rows
    #   xT:   [chunk, P, KH, TC]                         -> 8KB rows/chunk
    #   out:  [chunk, P, NH, TC]                         -> 8KB rows/chunk
    wq_d = nc.dram_tensor(
        "w_gu", [NS // 4, P, 4, KH, P], bf16, kind="ExternalInput"
    ).ap()
    wdn_d = nc.dram_tensor("w_dn", [P, KQ, H], bf16, kind="ExternalInput").ap()
    # chunks 1..7 of x; chunk 0 rides in the boot blob
    x_d = nc.dram_tensor(
        "xT", [NT - 1, P, KH, TC], bf16, kind="ExternalInput"
    ).ap()
    # boot blob: everything the first k-sweep needs, packed per partition as
    # [x0.k0-3 | slab0] and [x0.k4-7 | slab1] so ONE fat DMA per HWDGE queue
    # (6KB rows) pays the ~2us fixed cost once and lands it all together
    boot_d = nc.dram_tensor("boot", [2, P, BOOT], bf16, kind="ExternalInput").ap()
    rw_d = nc.dram_tensor("rw", [1, T], mybir.dt.float32, kind="ExternalInput").ap()
    # bf16 output: the host upcasts and sums the 8 expert partials in fp32;
    # the extra ~0.2% quantization is inside the 2e-2 budget and halves the
    # store traffic (shrinks the end-of-kernel DMA drain).
    out_d = nc.dram_tensor("out", [NT, P, NH, TC], bf16, kind="ExternalOutput").ap()

    from contextlib import ExitStack

    with tile.TileContext(nc) as tc:
        with ExitStack() as es:
            consts = es.enter_context(tc.tile_pool(name="consts", bufs=1))
            psum_gu = es.enter_context(tc.tile_pool(name="psum_gu", bufs=2, space="PSUM"))
            psum_o = es.enter_context(tc.tile_pool(name="psum_o", bufs=4, space="PSUM"))
            hid_pool = es.enter_context(tc.tile_pool(name="hid", bufs=2))
            tmp_pool = es.enter_context(tc.tile_pool(name="tmp", bufs=2))
            ost_pool = es.enter_context(tc.tile_pool(name="ost", bufs=2))
            wgu_s = consts.tile([P, NS, KH, P], bf16)
            wdn_s = consts.tile([P, KQ, H], bf16)
            # all x chunks stay SBUF-resident (62KB/partition with the boot
            # blob): no mid-stream x traffic at all after the startup loads
            x_s = consts.tile([P, NT - 1, KH, TC], bf16)
            boot_s = consts.tile([P, 2, BOOT], bf16)
            r_all = consts.tile([P, T], f32)

            def x0_mov(k):
                half, kk = divmod(k, 4)
                return boot_s[:, half, kk * TC:(kk + 1) * TC]

            def boot_slab(s, k):
                return boot_s[:, s, 4 * TC + k * P:4 * TC + (k + 1) * P]

            # PE p-state warmup: the engine idles from the end of the
            # framework prologue until the first weights+x land (~12.5us);
            # matmuls on a zeroed scratch tile during that window ramp the
            # clock so the real stream starts at full speed. Results land
            # in a PSUM bank that every real accumulation group resets
            # with start=True.
            dmy = consts.tile([P, 4, P], bf16)
            nc.gpsimd.memset(dmy, 0)
            warm_ps = psum_gu.tile([P, TC], f32, tag="gate")
            for _ in range(N_WARM):
                nc.tensor.matmul(
                    warm_ps, dmy[:, 0, :], dmy.rearrange("p a b -> p (a b)"),
                    start=True, stop=True,
                )

            # Startup schedule. The HBM aggregate (~320GB/s) is the binding
            # constraint and the queues round-robin for it, so: one boot-blob
            # DMA per HWDGE queue delivers the whole first k-sweep at once;
            # the rest of the weight stream (consumed at ~148GB/s by the PE)
            # is split 50/50 over the two fast queues as slab pairs in
            # consumption order; the slower SWDGE queue carries only traffic
            # whose deadline is far out, with x chunks 2-7 queued BEHIND
            # w_dn so they cannot steal bandwidth during the weight window.
            #   SP:    boot0 | pair45 | pair89   | pair12,13 | stores 0,2,4,6
            #   ACT:   boot1 | pair67 | pair10,11 | pair14,15
            #          | routing bcast | stores 1,3,5
            #   SWDGE: pair23 | w_dn | x1 | x2 | ... | x7
            nc.sync.dma_start(out=boot_s[:, 0], in_=boot_d[0])
            nc.scalar.dma_start(out=boot_s[:, 1], in_=boot_d[1])
            nc.gpsimd.dma_start(out=wgu_s[:, 2:4], in_=wq_d[0, :, 2:4])
            nc.sync.dma_start(out=wgu_s[:, 4:6], in_=wq_d[1, :, 0:2])
            nc.scalar.dma_start(out=wgu_s[:, 6:8], in_=wq_d[1, :, 2:4])
            nc.sync.dma_start(out=wgu_s[:, 8:10], in_=wq_d[2, :, 0:2])
            nc.scalar.dma_start(out=wgu_s[:, 10:12], in_=wq_d[2, :, 2:4])
            nc.sync.dma_start(out=wgu_s[:, 12:14], in_=wq_d[3, :, 0:2])
            nc.scalar.dma_start(out=wgu_s[:, 14:16], in_=wq_d[3, :, 2:4])
            nc.gpsimd.dma_start(out=wdn_s, in_=wdn_d)
            for c in range(1, NT):
                nc.gpsimd.dma_start(out=x_s[:, c - 1], in_=x_d[c - 1])
            nc.scalar.dma_start(out=r_all, in_=rw_d.to_broadcast([P, T]))

            for tci in range(NT):
                t0 = tci * TC

                def mov(k, tci=tci):
                    if tci == 0:
                        return x0_mov(k)
                    return x_s[:, tci - 1, k, :]

                def stat(s, k):
                    if s < 2:
                        # slabs 0/1 live in the boot blob, all chunks
                        return boot_slab(s, k)
                    return wgu_s[:, s, k, :]

                r_c = r_all[:, t0:t0 + TC]
                hid = hid_pool.tile([P, KQ, TC], bf16)
                for qi in range(KQ):
                    gate_ps = psum_gu.tile([P, TC], f32, tag="gate")
                    up_ps = psum_gu.tile([P, TC], f32, tag="up")
                    for k in range(KH):
                        nc.tensor.matmul(
                            gate_ps,
                            stat(2 * qi, k),
                            mov(k),
                            start=(k == 0),
                            stop=(k == KH - 1),
                        )
                    for k in range(KH):
                        nc.tensor.matmul(
                            up_ps,
                            stat(2 * qi + 1, k),
                            mov(k),
                            start=(k == 0),
                            stop=(k == KH - 1),
                        )
                    tmp = tmp_pool.tile([P, TC], f32)
                    nc.scalar.activation(
                        tmp, gate_ps, mybir.ActivationFunctionType.Silu
                    )
                    nc.vector.tensor_mul(hid[:, qi, :], tmp, up_ps)

                ost = ost_pool.tile([P, NH, TC], bf16, tag="ost")
                for hi in range(NH):
                    o_ps = psum_o.tile([P, TC], f32)
                    for qi in range(KQ):
                        nc.tensor.matmul(
                            o_ps,
                            wdn_s[:, qi, hi * P:(hi + 1) * P],
                            hid[:, qi, :],
                            start=(qi == 0),
                            stop=(qi == KQ - 1),
                        )
                    nc.vector.tensor_mul(ost[:, hi, :], o_ps, r_c)
                    if tci < NT - 1:
                        # one fat 1MB store per chunk (8KB rows), rings
                        # alternating so neither queue backs up
                        if hi == NH - 1:
                            eng = nc.sync if tci % 2 == 0 else nc.scalar
                            eng.dma_start(out=out_d[tci], in_=ost)
                    else:
                        # last chunk: split so the final piece after the last
                        # matmul is small and the two rings drain in parallel
                        if hi == 3:
                            nc.sync.dma_start(
                                out=out_d[tci, :, 0:4], in_=ost[:, 0:4]
                            )
                        elif hi == 6:
                            nc.scalar.dma_start(
                                out=out_d[tci, :, 4:7], in_=ost[:, 4:7]
                            )
                        elif hi == 7:
                            nc.sync.dma_start(
                                out=out_d[tci, :, 7:8], in_=ost[:, 7:8]
                            )
    _split_waits(nc)
    return nc


def _get_nc():
    global _CACHED
    if _CACHED is None:
        _CACHED = _build()
    return _CACHED


def _pack_wgu(w):
    """(H, 2Q) -> (4, P, 4, KH, P) bf16, quad-major in first-use slab order
    (gate qi / up qi interleaved), so a 4-slab quad reads 8KB contiguous per
    partition."""
    import ml_dtypes

    w = np.asarray(w, dtype=np.float32)
    # (KH, P, n_blk, P): k-tile, partition, column block, column
    w4 = w.reshape(KH, P, NS, P)
    order = [b for qi in range(KQ) for b in (qi, KQ + qi)]
    slabs = w4.transpose(2, 1, 0, 3)[order]          # (NS, P, KH, P)
    quads = slabs.reshape(NS // 4, 4, P, KH, P).transpose(0, 2, 1, 3, 4)
    return np.ascontiguousarray(quads.astype(ml_dtypes.bfloat16))


def _make_in_maps(x, routing_weights, gate_up_proj, down_proj):
    import ml_dtypes

    x = np.asarray(x, dtype=np.float32)
    # x[t, h] -> xP[chunk, p, k, t_in] with h = k*P + p: 8KB rows per chunk
    xP = x.reshape(NT, TC, KH, P).transpose(0, 3, 2, 1).astype(ml_dtypes.bfloat16)
    rw = np.asarray(routing_weights, dtype=np.float32)
    in_maps = []
    for e in range(E):
        dn = np.asarray(down_proj[e], dtype=np.float32)
        wq = _pack_wgu(gate_up_proj[e])
        # boot half s: [x0.k(4s..4s+3) flat | slab s], 6KB per partition
        boot = np.concatenate(
            [
                xP[0].reshape(P, 2, KH // 2 * TC).transpose(1, 0, 2),
                wq[0, :, 0:2].reshape(P, 2, KH * P).transpose(1, 0, 2),
            ],
            axis=2,
        )
        in_maps.append({
            "xT": np.ascontiguousarray(xP[1:]),
            "w_gu": wq,
            "boot": np.ascontiguousarray(boot),
            # w_dn[p, qi, h] = down_proj[qi*P + p, h]: 16KB rows
            "w_dn": np.ascontiguousarray(
                dn.reshape(KQ, P, H).transpose(1, 0, 2).astype(ml_dtypes.bfloat16)
            ),
            "rw": np.ascontiguousarray(rw[:, e].reshape(1, T)),
        })
    return in_maps


def _reduce_out(res):
    total = np.zeros((NT, P, NH, TC), dtype=np.float32)
    for r in res.results:
        total += r["out"].astype(np.float32).reshape(NT, P, NH, TC)
    # [chunk, p, hi, t_in] -> (T, H) with h = hi*P + p
    return np.ascontiguousarray(
        total.transpose(0, 3, 2, 1).reshape(T, H)
    )


def kernel(x, routing_weights, gate_up_proj, down_proj):
    from concourse.bass_utils import run_bass_kernel_spmd

    nc = _get_nc()
    in_maps = _make_in_maps(x, routing_weights, gate_up_proj, down_proj)
    res = run_bass_kernel_spmd(nc, in_maps, core_ids=list(range(E)))
    return _reduce_out(res)


# revision 38
# speedup vs baseline: 1.0839x; 1.0839x over previous
"""MoE experts kernel for TRN2, expert-parallel over 8 NeuronCores.

Reference computation (T=4096, E=8, H=1024, Q=1024):
    gate_up = einsum('th,ehq->teq', x, gate_up_proj)      # (T, E, 2Q)
    gate, up = split(gate_up, 2, axis=-1)
    hidden = silu(gate) * up                              # (T, E, Q)
    expert_outputs = einsum('teq,eqh->teh', hidden, down_proj)
    out = einsum('teh,te->th', expert_outputs, routing_weights)

Sharding: expert-parallel. Core e computes its expert's full contribution
r[:, e] * (silu(x @ Wgu_gate) * (x @ Wgu_up)) @ Wdn  for all T tokens,
entirely in feature-major layout (features on partitions, tokens on the
free axis) so no on-device transposes are needed; the host sums the 8
partial outputs (the expert-parallel all-reduce) and transposes back.

Per-core cost model (measured; ~352us total = 16.5 front + 331 stream
+ 5 drain):
  - 1536 bf16 matmuls of [128 contraction x 512 moving] at ~216ns each
    (512 cycles at the 2.37GHz sustained PE clock, zero per-instruction
    overhead -- bigger moving tiles would save nothing) = 332us PE busy.
    fp8 is no help twice over: its quantization error (3.8-6.5%
    end-to-end) eats the 2e-2 gate's margin, AND DoubleRow measures at
    most ~1.44x, so one residual-compensation matmul already makes it a
    net loss vs bf16.
  - Each dma_start costs ~2us fixed (completion receipt) + transfer,
    serialized per queue; per-queue throughput ~115GB/s warm, and the
    whole DMA subsystem is capped at ~200GB/s aggregate for the first
    ~10us regardless of queue count or descriptor sizes (measured
    identically across 2-way/3-way splits and fine/coarse pieces). So
    the 1.5MB the first k-sweep needs (x chunk 0 + slabs 0/1) cannot
    land before ~16.5us: prologue 7.6 + issue 0.7 + 1.5MB/200GB/s + 2
    fixed. The boot blob packs exactly that as ONE fat DMA per HWDGE
    queue; everything else is host-packed for 4-16KB contiguous rows
    and scheduled against the PE's consumption deadlines across the
    three queues (SP=nc.sync, ACT=nc.scalar, SWDGE=nc.gpsimd; SWDGE is
    slower and gets only far-deadline traffic).
  - The PE clock ramps over ~8 matmuls (~427ns each) after idling;
    dummy matmuls during the unavoidable initial DMA wait pay that cost
    off the critical path. A mid-stream starve >1us re-ramps (costly
    twice), so every weight pair must beat its deadline.
  - An external ~50-160ns hiccup hits the PE every 10.79us (~1.7us
    total, phase-random): unfixable, and the source of most run-to-run
    variance. Back-to-back runs throttle the chip to ~2.0GHz (matmul
    p50 259ns) -- cool ~2min between measurements.
  - ~7.5us fixed framework prologue before any DMA issues; ~1.8us
    epilogue barrier after the last store's semaphore. Drain floor
    ~4.9us: last mul + 2us DMA fixed + 128KB + epilogue (SWDGE for the
    final piece has +-2us latency variance; keep it on SP).
"""

import sys

for _p in ("/opt/trn_rl_repo", "/root/.axon_site/_ro/trn_rl_repo"):
    if _p not in sys.path:
        sys.path.insert(0, _p)

import numpy as np

T, E, H, Q = 4096, 8, 1024, 1024
P = 128          # partitions
TC = 512         # token chunk (= one PSUM bank of fp32)
NT = T // TC     # 8 token chunks
KH = H // P      # 8 contraction tiles for the gate_up matmul
KQ = Q // P      # 8 contraction tiles for the down matmul
NH = H // P      # 8 output-feature tiles
NS = 2 * Q // P  # 16 gate_up weight slabs (gate qi / up qi interleaved)
N_WARM = 18      # PE clock-warmup dummies covering the first-DMA wait
BOOT = KH * TC // 2 + KH * P  # 3072: half of x chunk 0 + one slab, per queue

_CACHED = None


def _split_waits(nc, max_waits=1):
    """Walrus codegen for several TRN2 ISA structs accepts only one sync-wait
    per instruction ("Too many sync wait commands"). Splitting is safe: a
    same-engine NoOp earlier in the (FIFO) stream carrying the extra waits
    blocks the stream at the same point the original multi-wait would have."""
    import concourse.mybir as mybir

    for f in nc.m.functions:
        for blk in f.blocks:
            newlist, changed = [], False
            for inst in blk.instructions:
                si = inst.sync_info
                if si is not None and si.on_wait and len(si.on_wait) > max_waits:
                    extra = si.on_wait[:-max_waits]
                    keep = si.on_wait[-max_waits:]
                    inst.sync_info = mybir.SyncInfo(
                        on_wait=list(keep), on_update=list(si.on_update or [])
                    )
                    for j, w in enumerate(extra):
                        nop = mybir.InstNoOp(
                            name=f"{inst.name}-wn{j}", engine=inst.engine
                        )
                        nop.sync_info = mybir.SyncInfo(on_wait=[w], on_update=[])
                        newlist.append(nop)
                    changed = True
                newlist.append(inst)
            if changed:
                blk.instructions = newlist


def _build():
    import concourse.bass as bass
    import concourse.mybir as mybir
    import concourse.tile as tile

    nc = bass.Bass("TRN2", target_bir_lowering=False, debug=False, num_devices=E)

    f32 = mybir.dt.float32
    # bf16: same PE rate as fp32r (1 cycle/row for moving >= 256) but half
    # the HBM traffic and half-width weight loads; quantization adds ~0.3%
    # relative error, well inside the 2e-2 gate.
    bf16 = mybir.dt.bfloat16

    # All DRAM layouts are host-packed so each DMA reads big contiguous
    # per-partition rows (see _make_in_maps):
    #   w_gu: quad-major [quad, P, slab-in-quad, KH, P]  -> 8KB rows/quad
    #   w_dn: [P, KQ, H]                                 -> 16KB rows
    #   xT:   [chunk, P, KH, TC]                         -> 8KB rows/chunk
    #   out:  [chunk, P, NH, TC]                         -> 8KB rows/chunk
    wq_d = nc.dram_tensor(
        "w_gu", [NS // 4, P, 4, KH, P], bf16, kind="ExternalInput"
    ).ap()
    wdn_d = nc.dram_tensor("w_dn", [P, KQ, H], bf16, kind="ExternalInput").ap()
    # chunks 1..7 of x; chunk 0 rides in the boot blob
    x_d = nc.dram_tensor(
        "xT", [NT - 1, P, KH, TC], bf16, kind="ExternalInput"
    ).ap()
    # boot blob: everything the first k-sweep needs, packed per partition as
    # [x0.k0-3 | slab0] and [x0.k4-7 | slab1] so ONE fat DMA per HWDGE queue
    # (6KB rows) pays the ~2us fixed cost once and lands it all together
    boot_d = nc.dram_tensor("boot", [2, P, BOOT], bf16, kind="ExternalInput").ap()
    rw_d = nc.dram_tensor("rw", [1, T], mybir.dt.float32, kind="ExternalInput").ap()
    # bf16 output: the host upcasts and sums the 8 expert partials in fp32;
    # the extra ~0.2% quantization is inside the 2e-2 budget and halves the
    # store traffic (shrinks the end-of-kernel DMA drain).
    out_d = nc.dram_tensor("out", [NT, P, NH, TC], bf16, kind="ExternalOutput").ap()

    from contextlib import ExitStack

    with tile.TileContext(nc) as tc:
        with ExitStack() as es:
            consts = es.enter_context(tc.tile_pool(name="consts", bufs=1))
            psum_gu = es.enter_context(tc.tile_pool(name="psum_gu", bufs=2, space="PSUM"))
            psum_o = es.enter_context(tc.tile_pool(name="psum_o", bufs=4, space="PSUM"))
            hid_pool = es.enter_context(tc.tile_pool(name="hid", bufs=2))
            tmp_pool = es.enter_context(tc.tile_pool(name="tmp", bufs=2))
            ost_pool = es.enter_context(tc.tile_pool(name="ost", bufs=2))
            wgu_s = consts.tile([P, NS, KH, P], bf16)
            wdn_s = consts.tile([P, KQ, H], bf16)
            # all x chunks stay SBUF-resident (62KB/partition with the boot
            # blob): no mid-stream x traffic at all after the startup loads
            x_s = consts.tile([P, NT - 1, KH, TC], bf16)
            boot_s = consts.tile([P, 2, BOOT], bf16)
            r_all = consts.tile([P, T], f32)

            def x0_mov(k):
                half, kk = divmod(k, 4)
                return boot_s[:, half, kk * TC:(kk + 1) * TC]

            def boot_slab(s, k):
                return boot_s[:, s, 4 * TC + k * P:4 * TC + (k + 1) * P]

            # PE p-state warmup: the engine idles from the end of the
            # framework prologue until the first weights+x land (~12.5us);
            # matmuls on a zeroed scratch tile during that window ramp the
            # clock so the real stream starts at full speed. Results land
            # in a PSUM bank that every real accumulation group resets
            # with start=True.
            dmy = consts.tile([P, 4, P], bf16)
            nc.gpsimd.memset(dmy, 0)
            warm_ps = psum_gu.tile([P, TC], f32, tag="gate")
            for _ in range(N_WARM):
                nc.tensor.matmul(
                    warm_ps, dmy[:, 0, :], dmy.rearrange("p a b -> p (a b)"),
                    start=True, stop=True,
                )

            # Startup schedule. The HBM aggregate (~320GB/s) is the binding
            # constraint and the queues round-robin for it, so: one boot-blob
            # DMA per HWDGE queue delivers the whole first k-sweep at once;
            # the rest of the weight stream (consumed at ~148GB/s by the PE)
            # is split 50/50 over the two fast queues as slab pairs in
            # consumption order; the slower SWDGE queue carries only traffic
            # whose deadline is far out, with the x chunks queued BEHIND
            # w_dn so they cannot steal bandwidth during the weight window.
            #   SP:    boot0 | pair45 | pair89   | pair12,13 | stores 0,2,4,6
            #   ACT:   boot1 | pair67 | pair10,11 | pair14,15
            #          | routing bcast | stores 1,3,5
            #   SWDGE: pair23 | w_dn | x1 | x2 | ... | x7
            nc.sync.dma_start(out=boot_s[:, 0], in_=boot_d[0])
            nc.scalar.dma_start(out=boot_s[:, 1], in_=boot_d[1])
            nc.gpsimd.dma_start(out=wgu_s[:, 2:4], in_=wq_d[0, :, 2:4])
            nc.sync.dma_start(out=wgu_s[:, 4:6], in_=wq_d[1, :, 0:2])
            nc.scalar.dma_start(out=wgu_s[:, 6:8], in_=wq_d[1, :, 2:4])
            nc.sync.dma_start(out=wgu_s[:, 8:10], in_=wq_d[2, :, 0:2])
            nc.scalar.dma_start(out=wgu_s[:, 10:12], in_=wq_d[2, :, 2:4])
            nc.sync.dma_start(out=wgu_s[:, 12:14], in_=wq_d[3, :, 0:2])
            nc.scalar.dma_start(out=wgu_s[:, 14:16], in_=wq_d[3, :, 2:4])
            nc.gpsimd.dma_start(out=wdn_s, in_=wdn_d)
            for c in range(1, NT):
                nc.gpsimd.dma_start(out=x_s[:, c - 1], in_=x_d[c - 1])
            nc.scalar.dma_start(out=r_all, in_=rw_d.to_broadcast([P, T]))

            for tci in range(NT):
                t0 = tci * TC

                def mov(k, tci=tci):
                    if tci == 0:
                        return x0_mov(k)
                    return x_s[:, tci - 1, k, :]

                def stat(s, k):
                    if s < 2:
                        # slabs 0/1 live in the boot blob, all chunks
                        return boot_slab(s, k)
                    return wgu_s[:, s, k, :]

                r_c = r_all[:, t0:t0 + TC]
                hid = hid_pool.tile([P, KQ, TC], bf16)
                for qi in range(KQ):
                    gate_ps = psum_gu.tile([P, TC], f32, tag="gate")
                    up_ps = psum_gu.tile([P, TC], f32, tag="up")
                    for k in range(KH):
                        nc.tensor.matmul(
                            gate_ps,
                            stat(2 * qi, k),
                            mov(k),
                            start=(k == 0),
                            stop=(k == KH - 1),
                        )
                    for k in range(KH):
                        nc.tensor.matmul(
                            up_ps,
                            stat(2 * qi + 1, k),
                            mov(k),
                            start=(k == 0),
                            stop=(k == KH - 1),
                        )
                    tmp = tmp_pool.tile([P, TC], f32)
                    nc.scalar.activation(
                        tmp, gate_ps, mybir.ActivationFunctionType.Silu
                    )
                    nc.vector.tensor_mul(hid[:, qi, :], tmp, up_ps)

                ost = ost_pool.tile([P, NH, TC], bf16, tag="ost")
                for hi in range(NH):
                    o_ps = psum_o.tile([P, TC], f32)
                    for qi in range(KQ):
                        nc.tensor.matmul(
                            o_ps,
                            wdn_s[:, qi, hi * P:(hi + 1) * P],
                            hid[:, qi, :],
                            start=(qi == 0),
                            stop=(qi == KQ - 1),
                        )
                    nc.vector.tensor_mul(ost[:, hi, :], o_ps, r_c)
                    if tci < NT - 1:
                        # one fat 1MB store per chunk (8KB rows), rings
                        # alternating so neither queue backs up
                        if hi == NH - 1:
                            eng = nc.sync if tci % 2 == 0 else nc.scalar
                            eng.dma_start(out=out_d[tci], in_=ost)
                    else:
                        # last chunk: split so the final piece after the last
                        # matmul is small and the two rings drain in parallel
                        if hi == 3:
                            nc.sync.dma_start(
                                out=out_d[tci, :, 0:4], in_=ost[:, 0:4]
                            )
                        elif hi == 6:
                            nc.scalar.dma_start(
                                out=out_d[tci, :, 4:7], in_=ost[:, 4:7]
                            )
                        elif hi == 7:
                            nc.sync.dma_start(
                                out=out_d[tci, :, 7:8], in_=ost[:, 7:8]
                            )
    _split_waits(nc)
    return nc


def _get_nc():
    global _CACHED
    if _CACHED is None:
        _CACHED = _build()
    return _CACHED


def _pack_wgu(w):
    """(H, 2Q) -> (4, P, 4, KH, P) bf16, quad-major in first-use slab order
    (gate qi / up qi interleaved), so a 4-slab quad reads 8KB contiguous per
    partition."""
    import ml_dtypes

    w = np.asarray(w, dtype=np.float32)
    # (KH, P, n_blk, P): k-tile, partition, column block, column
    w4 = w.reshape(KH, P, NS, P)
    order = [b for qi in range(KQ) for b in (qi, KQ + qi)]
    slabs = w4.transpose(2, 1, 0, 3)[order]          # (NS, P, KH, P)
    quads = slabs.reshape(NS // 4, 4, P, KH, P).transpose(0, 2, 1, 3, 4)
    return np.ascontiguousarray(quads.astype(ml_dtypes.bfloat16))


def _make_in_maps(x, routing_weights, gate_up_proj, down_proj):
    import ml_dtypes

    x = np.asarray(x, dtype=np.float32)
    # x[t, h] -> xP[chunk, p, k, t_in] with h = k*P + p: 8KB rows per chunk
    xP = x.reshape(NT, TC, KH, P).transpose(0, 3, 2, 1).astype(ml_dtypes.bfloat16)
    rw = np.asarray(routing_weights, dtype=np.float32)
    in_maps = []
    for e in range(E):
        dn = np.asarray(down_proj[e], dtype=np.float32)
        wq = _pack_wgu(gate_up_proj[e])
        # boot half s: [x0.k(4s..4s+3) flat | slab s], 6KB per partition
        boot = np.concatenate(
            [
                xP[0].reshape(P, 2, KH // 2 * TC).transpose(1, 0, 2),
                wq[0, :, 0:2].reshape(P, 2, KH * P).transpose(1, 0, 2),
            ],
            axis=2,
        )
        in_maps.append({
            "xT": np.ascontiguousarray(xP[1:]),
            "w_gu": wq,
            "boot": np.ascontiguousarray(boot),
            # w_dn[p, qi, h] = down_proj[qi*P + p, h]: 16KB rows
            "w_dn": np.ascontiguousarray(
                dn.reshape(KQ, P, H).transpose(1, 0, 2).astype(ml_dtypes.bfloat16)
            ),
            "rw": np.ascontiguousarray(rw[:, e].reshape(1, T)),
        })
    return in_maps


def _reduce_out(res):
    total = np.zeros((NT, P, NH, TC), dtype=np.float32)
    for r in res.results:
        total += r["out"].astype(np.float32).reshape(NT, P, NH, TC)
    # [chunk, p, hi, t_in] -> (T, H) with h = hi*P + p
    return np.ascontiguousarray(
        total.transpose(0, 3, 2, 1).reshape(T, H)
    )


def kernel(x, routing_weights, gate_up_proj, down_proj):
    from concourse.bass_utils import run_bass_kernel_spmd

    nc = _get_nc()
    in_maps = _make_in_maps(x, routing_weights, gate_up_proj, down_proj)
    res = run_bass_kernel_spmd(nc, in_maps, core_ids=list(range(E)))
    return _reduce_out(res)


# revision 44
# speedup vs baseline: 1.0907x; 1.0063x over previous
"""MoE experts kernel for TRN2, expert-parallel over 8 NeuronCores.

Reference computation (T=4096, E=8, H=1024, Q=1024):
    gate_up = einsum('th,ehq->teq', x, gate_up_proj)      # (T, E, 2Q)
    gate, up = split(gate_up, 2, axis=-1)
    hidden = silu(gate) * up                              # (T, E, Q)
    expert_outputs = einsum('teq,eqh->teh', hidden, down_proj)
    out = einsum('teh,te->th', expert_outputs, routing_weights)

Sharding: expert-parallel. Core e computes its expert's full contribution
r[:, e] * (silu(x @ Wgu_gate) * (x @ Wgu_up)) @ Wdn  for all T tokens,
entirely in feature-major layout (features on partitions, tokens on the
free axis) so no on-device transposes are needed; the host sums the 8
partial outputs (the expert-parallel all-reduce) and transposes back.

Per-core cost model (measured; ~352us total = 16.5 front + 331 stream
+ 5 drain):
  - 1536 bf16 matmuls of [128 contraction x 512 moving] at ~216ns each
    (512 cycles at the 2.37GHz sustained PE clock, zero per-instruction
    overhead -- bigger moving tiles would save nothing) = 332us PE busy.
    fp8 is no help twice over: its quantization error (3.8-6.5%
    end-to-end) eats the 2e-2 gate's margin, AND DoubleRow measures at
    most ~1.44x, so one residual-compensation matmul already makes it a
    net loss vs bf16.
  - Each dma_start costs ~2us fixed (completion receipt) + transfer,
    serialized per queue; per-queue throughput ~115GB/s warm, and the
    whole DMA subsystem is capped at ~200GB/s aggregate for the first
    ~10us regardless of queue count or descriptor sizes (measured
    identically across 2-way/3-way splits and fine/coarse pieces). So
    the 1.5MB the first k-sweep needs (x chunk 0 + slabs 0/1) cannot
    land before ~16.5us: prologue 7.6 + issue 0.7 + 1.5MB/200GB/s + 2
    fixed. The boot blob packs exactly that as ONE fat DMA per HWDGE
    queue; everything else is host-packed for 4-16KB contiguous rows
    and scheduled against the PE's consumption deadlines across the
    three queues (SP=nc.sync, ACT=nc.scalar, SWDGE=nc.gpsimd; SWDGE is
    slower and gets only far-deadline traffic).
  - The PE clock ramps over ~8 matmuls (~427ns each) after idling;
    dummy matmuls during the unavoidable initial DMA wait pay that cost
    off the critical path. A mid-stream starve >1us re-ramps (costly
    twice), so every weight pair must beat its deadline.
  - An external ~50-160ns hiccup hits the PE every 10.79us (~1.7us
    total, phase-random): unfixable, and the source of most run-to-run
    variance. Back-to-back runs throttle the chip to ~2.0GHz (matmul
    p50 259ns) -- cool ~2min between measurements.
  - ~7.5us fixed framework prologue before any DMA issues; ~1.8us
    epilogue barrier after the last store's semaphore. Drain floor
    ~4.9us: last mul + 2us DMA fixed + 128KB + epilogue (SWDGE for the
    final piece has +-2us latency variance; keep it on SP).
"""

import sys

for _p in ("/opt/trn_rl_repo", "/root/.axon_site/_ro/trn_rl_repo"):
    if _p not in sys.path:
        sys.path.insert(0, _p)

import numpy as np

T, E, H, Q = 4096, 8, 1024, 1024
P = 128          # partitions
TC = 512         # token chunk (= one PSUM bank of fp32)
NT = T // TC     # 8 token chunks
KH = H // P      # 8 contraction tiles for the gate_up matmul
KQ = Q // P      # 8 contraction tiles for the down matmul
NH = H // P      # 8 output-feature tiles
NS = 2 * Q // P  # 16 gate_up weight slabs (gate qi / up qi interleaved)
N_WARM = 15      # PE clock-warmup dummies covering the first-DMA wait
# boot half, in bytes: half of x chunk 0 as fp8 + one slab as bf16
BOOT_B = KH * TC // 2 + KH * P * 2  # 4096

_CACHED = None


def _split_waits(nc, max_waits=1):
    """Walrus codegen for several TRN2 ISA structs accepts only one sync-wait
    per instruction ("Too many sync wait commands"). Splitting is safe: a
    same-engine NoOp earlier in the (FIFO) stream carrying the extra waits
    blocks the stream at the same point the original multi-wait would have."""
    import concourse.mybir as mybir

    for f in nc.m.functions:
        for blk in f.blocks:
            newlist, changed = [], False
            for inst in blk.instructions:
                si = inst.sync_info
                if si is not None and si.on_wait and len(si.on_wait) > max_waits:
                    extra = si.on_wait[:-max_waits]
                    keep = si.on_wait[-max_waits:]
                    inst.sync_info = mybir.SyncInfo(
                        on_wait=list(keep), on_update=list(si.on_update or [])
                    )
                    for j, w in enumerate(extra):
                        nop = mybir.InstNoOp(
                            name=f"{inst.name}-wn{j}", engine=inst.engine
                        )
                        nop.sync_info = mybir.SyncInfo(on_wait=[w], on_update=[])
                        newlist.append(nop)
                    changed = True
                newlist.append(inst)
            if changed:
                blk.instructions = newlist


def _build():
    import concourse.bass as bass
    import concourse.mybir as mybir
    import concourse.tile as tile

    nc = bass.Bass("TRN2", target_bir_lowering=False, debug=False, num_devices=E)

    f32 = mybir.dt.float32
    # bf16: same PE rate as fp32r (1 cycle/row for moving >= 256) but half
    # the HBM traffic and half-width weight loads; quantization adds ~0.3%
    # relative error, well inside the 2e-2 gate.
    bf16 = mybir.dt.bfloat16

    # All DRAM layouts are host-packed so each DMA reads big contiguous
    # per-partition rows (see _make_in_maps):
    #   w_gu: quad-major [quad, P, slab-in-quad, KH, P]  -> 8KB rows/quad
    #   w_dn: [P, KQ, H]                                 -> 16KB rows
    #   xT:   [chunk, P, KH, TC]                         -> 8KB rows/chunk
    #   out:  [chunk, P, NH, TC]                         -> 8KB rows/chunk
    wq_d = nc.dram_tensor(
        "w_gu", [NS // 4, P, 4, KH, P], bf16, kind="ExternalInput"
    ).ap()
    wdn_d = nc.dram_tensor("w_dn", [P, KQ, H], bf16, kind="ExternalInput").ap()
    # chunks 1..7 of x; chunk 0 rides in the boot blob
    x_d = nc.dram_tensor(
        "xT", [NT - 1, P, KH, TC], bf16, kind="ExternalInput"
    ).ap()
    # boot blob: everything the first k-sweep needs, packed per partition as
    # [x0.k0-3 fp8 | slab0 bf16] and [x0.k4-7 fp8 | slab1 bf16] so ONE fat
    # 4KB-row DMA per HWDGE queue pays the ~2us fixed cost once and lands it
    # all together. x rides as fp8 (x~N(0,1) quantizes cleanly unscaled;
    # chunk-0-only quantization moves end-to-end error 4.4e-3 -> ~8e-3, still
    # 2.5x under the 2e-2 gate) and is upcast to bf16 on the idle vector and
    # scalar engines, pipelined ahead of the PE's first k-sweep: 1MB of boot
    # instead of 1.5MB through the ~200GB/s cold-DMA window.
    boot_d = nc.dram_tensor(
        "boot", [2, P, BOOT_B], mybir.dt.uint8, kind="ExternalInput"
    ).ap()
    rw_d = nc.dram_tensor("rw", [1, T], mybir.dt.float32, kind="ExternalInput").ap()
    # bf16 output: the host upcasts and sums the 8 expert partials in fp32;
    # the extra ~0.2% quantization is inside the 2e-2 budget and halves the
    # store traffic (shrinks the end-of-kernel DMA drain).
    out_d = nc.dram_tensor("out", [NT, P, NH, TC], bf16, kind="ExternalOutput").ap()

    from contextlib import ExitStack

    with tile.TileContext(nc) as tc:
        with ExitStack() as es:
            consts = es.enter_context(tc.tile_pool(name="consts", bufs=1))
            psum_gu = es.enter_context(tc.tile_pool(name="psum_gu", bufs=2, space="PSUM"))
            psum_o = es.enter_context(tc.tile_pool(name="psum_o", bufs=4, space="PSUM"))
            hid_pool = es.enter_context(tc.tile_pool(name="hid", bufs=2))
            tmp_pool = es.enter_context(tc.tile_pool(name="tmp", bufs=2))
            ost_pool = es.enter_context(tc.tile_pool(name="ost", bufs=2))
            wgu_s = consts.tile([P, NS, KH, P], bf16)
            wdn_s = consts.tile([P, KQ, H], bf16)
            # all x chunks stay SBUF-resident (62KB/partition with the boot
            # blob): no mid-stream x traffic at all after the startup loads
            x_s = consts.tile([P, NT - 1, KH, TC], bf16)
            boot_s = consts.tile([P, 2, BOOT_B], mybir.dt.uint8)
            x0b = consts.tile([P, KH, TC], bf16)
            r_all = consts.tile([P, T], f32)

            def x0_mov(k):
                return x0b[:, k, :]

            def boot_slab(s, k):
                return boot_s[
                    :, s, 4 * TC + 2 * k * P:4 * TC + 2 * (k + 1) * P
                ].bitcast(bf16)

            # PE p-state warmup: the engine idles from the end of the
            # framework prologue until the first weights+x land (~12.5us);
            # matmuls on a zeroed scratch tile during that window ramp the
            # clock so the real stream starts at full speed. Results land
            # in a PSUM bank that every real accumulation group resets
            # with start=True.
            dmy = consts.tile([P, 4, P], bf16)
            nc.gpsimd.memset(dmy, 0)
            warm_ps = psum_gu.tile([P, TC], f32, tag="gate")
            for _ in range(N_WARM):
                nc.tensor.matmul(
                    warm_ps, dmy[:, 0, :], dmy.rearrange("p a b -> p (a b)"),
                    start=True, stop=True,
                )

            # Startup schedule. The HBM aggregate (~320GB/s) is the binding
            # constraint and the queues round-robin for it, so: one boot-blob
            # DMA per HWDGE queue delivers the whole first k-sweep at once;
            # the rest of the weight stream (consumed at ~148GB/s by the PE)
            # is split 50/50 over the two fast queues as slab pairs in
            # consumption order; the slower SWDGE queue carries only traffic
            # whose deadline is far out, with the x chunks queued BEHIND
            # w_dn so they cannot steal bandwidth during the weight window.
            #   SP:    boot0 | pair45 | pair89   | pair12,13 | stores 0,2,4,6
            #   ACT:   boot1 | pair67 | pair10,11 | pair14,15
            #          | routing bcast | stores 1,3,5
            #   SWDGE: pair23 | w_dn | x1 | x2 | ... | x7
            nc.sync.dma_start(out=boot_s[:, 0], in_=boot_d[0])
            nc.scalar.dma_start(out=boot_s[:, 1], in_=boot_d[1])
            nc.gpsimd.dma_start(out=wgu_s[:, 2:4], in_=wq_d[0, :, 2:4])
            nc.sync.dma_start(out=wgu_s[:, 4:6], in_=wq_d[1, :, 0:2])
            nc.scalar.dma_start(out=wgu_s[:, 6:8], in_=wq_d[1, :, 2:4])
            nc.sync.dma_start(out=wgu_s[:, 8:10], in_=wq_d[2, :, 0:2])
            nc.scalar.dma_start(out=wgu_s[:, 10:12], in_=wq_d[2, :, 2:4])
            nc.sync.dma_start(out=wgu_s[:, 12:14], in_=wq_d[3, :, 0:2])
            nc.scalar.dma_start(out=wgu_s[:, 14:16], in_=wq_d[3, :, 2:4])
            nc.gpsimd.dma_start(out=wdn_s, in_=wdn_d)
            for c in range(1, NT):
                nc.gpsimd.dma_start(out=x_s[:, c - 1], in_=x_d[c - 1])
            nc.scalar.dma_start(out=r_all, in_=rw_d.to_broadcast([P, T]))

            # fp8 -> bf16 upcast of x chunk 0, one k-tile per instruction so
            # the PE's ramped k-sweep (~430ns/tile) never waits: vector takes
            # k0-3, scalar (free once its DMA issues are done) takes k4-7
            for k in range(KH):
                half, kk = divmod(k, 4)
                src = boot_s[:, half, kk * TC:(kk + 1) * TC].bitcast(
                    mybir.dt.float8e4
                )
                if k < 4:
                    nc.vector.tensor_scalar_mul(x0b[:, k, :], src, 1.0)
                else:
                    nc.scalar.copy(x0b[:, k, :], src)

            for tci in range(NT):
                t0 = tci * TC

                def mov(k, tci=tci):
                    if tci == 0:
                        return x0_mov(k)
                    return x_s[:, tci - 1, k, :]

                def stat(s, k):
                    if s < 2:
                        # slabs 0/1 live in the boot blob, all chunks
                        return boot_slab(s, k)
                    return wgu_s[:, s, k, :]

                r_c = r_all[:, t0:t0 + TC]
                hid = hid_pool.tile([P, KQ, TC], bf16)
                for qi in range(KQ):
                    gate_ps = psum_gu.tile([P, TC], f32, tag="gate")
                    up_ps = psum_gu.tile([P, TC], f32, tag="up")
                    for k in range(KH):
                        nc.tensor.matmul(
                            gate_ps,
                            stat(2 * qi, k),
                            mov(k),
                            start=(k == 0),
                            stop=(k == KH - 1),
                        )
                    for k in range(KH):
                        nc.tensor.matmul(
                            up_ps,
                            stat(2 * qi + 1, k),
                            mov(k),
                            start=(k == 0),
                            stop=(k == KH - 1),
                        )
                    tmp = tmp_pool.tile([P, TC], f32)
                    nc.scalar.activation(
                        tmp, gate_ps, mybir.ActivationFunctionType.Silu
                    )
                    nc.vector.tensor_mul(hid[:, qi, :], tmp, up_ps)

                ost = ost_pool.tile([P, NH, TC], bf16, tag="ost")
                for hi in range(NH):
                    o_ps = psum_o.tile([P, TC], f32)
                    for qi in range(KQ):
                        nc.tensor.matmul(
                            o_ps,
                            wdn_s[:, qi, hi * P:(hi + 1) * P],
                            hid[:, qi, :],
                            start=(qi == 0),
                            stop=(qi == KQ - 1),
                        )
                    nc.vector.tensor_mul(ost[:, hi, :], o_ps, r_c)
                    if tci < NT - 1:
                        # one fat 1MB store per chunk (8KB rows), rings
                        # alternating so neither queue backs up
                        if hi == NH - 1:
                            eng = nc.sync if tci % 2 == 0 else nc.scalar
                            eng.dma_start(out=out_d[tci], in_=ost)
                    else:
                        # last chunk: split so the final piece after the last
                        # matmul is small and the two rings drain in parallel
                        if hi == 3:
                            nc.sync.dma_start(
                                out=out_d[tci, :, 0:4], in_=ost[:, 0:4]
                            )
                        elif hi == 6:
                            nc.scalar.dma_start(
                                out=out_d[tci, :, 4:7], in_=ost[:, 4:7]
                            )
                        elif hi == 7:
                            nc.sync.dma_start(
                                out=out_d[tci, :, 7:8], in_=ost[:, 7:8]
                            )
    _split_waits(nc)
    return nc


def _get_nc():
    global _CACHED
    if _CACHED is None:
        _CACHED = _build()
    return _CACHED


def _pack_wgu(w):
    """(H, 2Q) -> (4, P, 4, KH, P) bf16, quad-major in first-use slab order
    (gate qi / up qi interleaved), so a 4-slab quad reads 8KB contiguous per
    partition."""
    import ml_dtypes

    w = np.asarray(w, dtype=np.float32)
    # (KH, P, n_blk, P): k-tile, partition, column block, column
    w4 = w.reshape(KH, P, NS, P)
    order = [b for qi in range(KQ) for b in (qi, KQ + qi)]
    slabs = w4.transpose(2, 1, 0, 3)[order]          # (NS, P, KH, P)
    quads = slabs.reshape(NS // 4, 4, P, KH, P).transpose(0, 2, 1, 3, 4)
    return np.ascontiguousarray(quads.astype(ml_dtypes.bfloat16))


def _make_in_maps(x, routing_weights, gate_up_proj, down_proj):
    import ml_dtypes

    x = np.asarray(x, dtype=np.float32)
    # x[t, h] -> xP[chunk, p, k, t_in] with h = k*P + p: 8KB rows per chunk
    xPf = x.reshape(NT, TC, KH, P).transpose(0, 3, 2, 1)
    xP = xPf.astype(ml_dtypes.bfloat16)
    x0f32 = np.ascontiguousarray(xPf[0])
    rw = np.asarray(routing_weights, dtype=np.float32)
    in_maps = []
    for e in range(E):
        dn = np.asarray(down_proj[e], dtype=np.float32)
        wq = _pack_wgu(gate_up_proj[e])
        # boot half s: [x0.k(4s..4s+3) fp8 | slab s bf16] as raw bytes,
        # 4KB per partition (x quantized from the original fp32)
        x0_fp8 = x0f32.astype(ml_dtypes.float8_e4m3fn).view(np.uint8)
        boot = np.concatenate(
            [
                x0_fp8.reshape(P, 2, KH // 2 * TC).transpose(1, 0, 2),
                np.ascontiguousarray(wq[0, :, 0:2])
                .view(np.uint8)
                .reshape(P, 2, KH * P * 2)
                .transpose(1, 0, 2),
            ],
            axis=2,
        )
        in_maps.append({
            "xT": np.ascontiguousarray(xP[1:]),
            "w_gu": wq,
            "boot": np.ascontiguousarray(boot),
            # w_dn[p, qi, h] = down_proj[qi*P + p, h]: 16KB rows
            "w_dn": np.ascontiguousarray(
                dn.reshape(KQ, P, H).transpose(1, 0, 2).astype(ml_dtypes.bfloat16)
            ),
            "rw": np.ascontiguousarray(rw[:, e].reshape(1, T)),
        })
    return in_maps


def _reduce_out(res):
    total = np.zeros((NT, P, NH, TC), dtype=np.float32)
    for r in res.results:
        total += r["out"].astype(np.float32).reshape(NT, P, NH, TC)
    # [chunk, p, hi, t_in] -> (T, H) with h = hi*P + p
    return np.ascontiguousarray(
        total.transpose(0, 3, 2, 1).reshape(T, H)
    )


def kernel(x, routing_weights, gate_up_proj, down_proj):
    from concourse.bass_utils import run_bass_kernel_spmd

    nc = _get_nc()
    in_maps = _make_in_maps(x, routing_weights, gate_up_proj, down_proj)
    res = run_bass_kernel_spmd(nc, in_maps, core_ids=list(range(E)))
    return _reduce_out(res)
